# revision 41
# baseline (speedup 1.0000x reference)
"""HGCN encoder forward on 8 Trainium2 NeuronCores.

Computation (per batch b):
    w_abs = |gelu(states @ W1.T + b1) @ W2.T + b2|          (E,)  [host, tiny]
    d[n]    = sum_e H[n,e] * w_abs[e]                        (N,)  [host]
    s[n,dd] = rsqrt(d[n]) * nf[n,dd]                         [host, 2 MiB]
    Y[dd,e] = sum_n s[n,dd] * H[n,e]                         [device]
    X[e,dd] = leaky_relu(w_abs[e] * (Y_half0 + Y_half1)[dd,e])  [host, tiny]

Sharding: core c -> (batch b = c//2, node-half c%2); each core owns
4096 node rows.  The device kernel is a pure DMA->PE stream, paced
entirely by the H read:

  * H is centered (H - 0.5 in [-0.5, 0.5)) and quantized to fp8 E3M4
    on the host.  In [-0.5, 0.5) e3m4's subnormal+first-normal bands
    give a uniform ~6-bit quantizer (max err 2^-7); measured output
    rel err ~7e-3 vs the 2e-2 gate.  The removed mean re-enters as a
    host-side rank-1 correction: Y += 0.5 * colsum(s).
  * 8.4 MiB/core streams via 9 chunk DMAs (~1 MiB each; zero deps, all
    chunks SBUF-resident, no recycling) split alternately across BOTH
    HWDGE rings (Sync + Scalar issuers) -- measured ~410 GB/s, vs
    ~345 GB/s on a single ring.  ~1 MiB chunks matter twice: per-
    partition lines stay >= 4 KiB (descriptor efficiency) and chunk
    completion stays fine-grained so the PE paces tightly.  More
    chunks regress: each dma_start costs ~700 ns of issue time and
    the ring holds only ~10 in-flight DMAs.
  * PE: per node-tile, the 4 e-chunk matmuls (K=128, M=16, N=512,
    lhsT = bf16 s-tile, rhs = fp8 H) are column-tiled to 32-col strips
    (tile_position=(0,32j)) so they run concurrently in the array
    (measured 4 ns start-to-start); the PE tracks the DMA with ~3x
    headroom.  Accumulators sit in 4 separate PSUM banks (a start=True
    matmul clears its whole bank, so groups may never share one).
  * DVE/ACT only drain PSUM at the end (same-partition copies into a
    bf16 [128,512] output staging tile, alternating engines across
    banks); the output ships as two 64-partition bf16 DMAs (16 SDMA
    engines vs 2 for a [16,E] layout), one per ring, one per copy
    round.  bf16 output adds ~3e-4 rel err (7.03e-3 -> 7.33e-3) and
    halves the exposed output-transfer time.

Host sums the two per-batch partials, adds the mean correction, scales
by w_abs, applies leaky_relu.  Measured ~36.6 us/core quiet (ambient
HBM dips give 40-44) vs the 126 us baseline; span = ~8 us fixed
preamble + ~21 us stream + ~4.5 us drain tail + ~2.7 us postamble.
"""

import sys

for _p in ("/opt/trn_rl_repo",):
    if _p not in sys.path:
        sys.path.insert(0, _p)

import numpy as np

B, N, E, S, D = 4, 8192, 2048, 64, 16
NCORES = 8
NSHARD = N // 2          # nodes per core
NT = NSHARD // 128       # 32 node-tiles per core
ECH = 512                # e-chunk per matmul (one PSUM bank)
NJ = E // ECH            # 4 matmuls (banks) per node-tile
# Chunking sweet spot (measured): ~1 MiB chunks pace the PE tightly
# (fine completion granularity) while keeping the dma_start count
# inside the HWDGE ring depth; issues alternate between the two HWDGE
# rings (Sync + Scalar engines) so they don't serialize on one engine.
CHUNK_TILES = [4, 4, 4, 4, 4, 4, 4, 2, 2]
assert sum(CHUNK_TILES) == NT
COLTILE = True           # concurrent 32-col-strip matmuls

_CACHE = {}


def _build_nc():
    import concourse.bass as bass  # noqa: F401
    import concourse.mybir as mybir
    import concourse.tile as tile
    from concourse import bacc

    f32 = mybir.dt.float32
    bf16 = mybir.dt.bfloat16
    f8 = mybir.dt.float8e3
    nc = bacc.Bacc(
        "TRN2",
        target_bir_lowering=False,
        debug=False,
        num_devices=NCORES,
    )
    hg = nc.dram_tensor("hg", [128, NT * E], f8, kind="ExternalInput").ap()
    sv = nc.dram_tensor("sv", [128, NT * D], bf16, kind="ExternalInput").ap()
    # output packed [128, ECH]: bank j's [D, ECH] block lives at
    # partitions 32j..32j+16 (same partitions the column-tiled matmul
    # wrote, so the drain copies never shift partitions -- shifts must
    # be 32-aligned), and the output DMA spans all 128 partitions
    # (16 SDMA engines) instead of 16 (2 engines, ~4x slower).  Rows
    # 32j+16..32j+32 are dead weight the host ignores.
    y = nc.dram_tensor("y", [128, ECH], bf16, kind="ExternalOutput").ap()

    with tile.TileContext(nc) as tc:
        with (
            tc.tile_pool(name="hpool", bufs=1) as hpool,
            tc.tile_pool(name="wpool", bufs=1) as wpool,
            tc.tile_pool(name="psum", bufs=1, space="PSUM") as psum_pool,
        ):
            s_all = wpool.tile([128, NT * D], bf16, tag="sall")
            y_tile = wpool.tile([128, ECH], bf16, tag="y")
            # clear once so the output DMA never reads uninitialized
            # rows (runs in the preamble shadow, off the critical path)
            nc.vector.memset(y_tile[:], 0.0)
            nc.sync.dma_start(s_all[:], sv[:])

            accs = [
                psum_pool.tile([128, ECH], f32, tag=f"acc{j}", name=f"acc{j}")
                for j in range(NJ)
            ]

            chunks = []
            base = 0
            issuers = [nc.sync, nc.scalar]
            for c, ctiles in enumerate(CHUNK_TILES):
                h_c = hpool.tile([128, ctiles * E], f8, tag=f"hg{c}")
                eng = issuers[c % 2]
                eng.dma_start(h_c[:], hg[:, base * E : (base + ctiles) * E])
                chunks.append((h_c, base, ctiles))
                base += ctiles

            for h_c, base, ctiles in chunks:
                for t in range(ctiles):
                    i = base + t
                    for j in range(NJ):
                        if COLTILE:
                            out_ap = accs[j][32 * j : 32 * j + D, :]
                            tp = (0, 32 * j)
                        else:
                            out_ap = accs[j][0:D, :]
                            tp = None
                        nc.tensor.matmul(
                            out_ap,
                            lhsT=s_all[:, i * D : (i + 1) * D],
                            rhs=h_c[:, t * E + j * ECH : t * E + (j + 1) * ECH],
                            start=(i == 0),
                            stop=(i == NT - 1),
                            tile_position=tp,
                        )
                        if i == NT - 1:
                            rows = slice(32 * j, 32 * j + D)
                            src = (
                                accs[j][32 * j : 32 * j + D, :]
                                if COLTILE
                                else accs[j][0:D, :]
                            )
                            if j % 2 == 0:
                                nc.scalar.copy(y_tile[rows, :], src)
                            else:
                                nc.vector.tensor_copy(y_tile[rows, :], src)
                                # ship each copy-round's bank pair as one
                                # 64-partition DMA, alternating rings
                                pr = slice(32 * (j - 1), 32 * (j + 1))
                                eng = issuers[(j // 2) % 2]
                                eng.dma_start(y[pr, :], y_tile[pr, :])

    nc.compile()
    return nc


def _get_nc():
    if "nc" not in _CACHE:
        _CACHE["nc"] = _build_nc()
    return _CACHE["nc"]


def _host_wabs(states, W1, b1, W2, b2):
    from scipy.special import erf

    st = states.astype(np.float64)
    h = st @ W1.astype(np.float64).T + b1.astype(np.float64)
    h = h * 0.5 * (1.0 + erf(h / np.sqrt(2.0)))
    w = h @ W2.astype(np.float64).T + b2.astype(np.float64)
    return np.abs(w).astype(np.float32)  # (B, E)


def _f32_to_bf16_u16(x):
    """Round-to-nearest-even f32 -> bf16, returned as a uint16 array."""
    u = np.ascontiguousarray(x, dtype=np.float32).view(np.uint32)
    r = ((u >> 16) & 1) + np.uint32(0x7FFF)
    return ((u + r) >> 16).astype(np.uint16)


def _make_in_maps(node_features, hyper_graph, w_abs):
    import ml_dtypes

    # degree + rsqrt + row-scale of node features, all on host (exact)
    s = np.empty((B, N, D), dtype=np.float32)
    for b in range(B):
        d = hyper_graph[b] @ w_abs[b]                      # (N,)
        dinv = np.where(
            d > 0, 1.0 / np.sqrt(d.astype(np.float64)), 0.0
        ).astype(np.float32)
        s[b] = dinv[:, None] * node_features[b]

    s_u16 = _f32_to_bf16_u16(s)
    # mean-correction uses the bf16-rounded s the device actually sees
    s_bf = s_u16.view(ml_dtypes.bfloat16).astype(np.float32)  # (B,N,D)
    s_sum = s_bf.sum(axis=1)                                  # (B,D)

    hq = (hyper_graph - np.float32(0.5)).astype(ml_dtypes.float8_e3m4)

    in_maps = []
    for c in range(NCORES):
        b, half = c // 2, c % 2
        sl = slice(half * NSHARD, (half + 1) * NSHARD)
        hg_c = np.ascontiguousarray(
            hq[b, sl].view(np.uint8).reshape(NT, 128, E).transpose(1, 0, 2)
        ).reshape(128, NT * E).view(ml_dtypes.float8_e3m4)
        s_c = np.ascontiguousarray(
            s_u16[b, sl].reshape(NT, 128, D).transpose(1, 0, 2)
        ).reshape(128, NT * D).view(ml_dtypes.bfloat16)
        in_maps.append({"hg": hg_c, "sv": s_c})
    return in_maps, s_sum


def kernel(**inputs):
    from concourse.bass_utils import run_bass_kernel_spmd

    node_features = np.asarray(inputs["node_features"], dtype=np.float32)
    hyper_graph = np.asarray(inputs["hyper_graph"], dtype=np.float32)
    states = np.asarray(inputs["states"], dtype=np.float32)
    W1 = np.asarray(inputs["W1"], dtype=np.float32)
    b1 = np.asarray(inputs["b1"], dtype=np.float32)
    W2 = np.asarray(inputs["W2"], dtype=np.float32)
    b2 = np.asarray(inputs["b2"], dtype=np.float32)

    w_abs = _host_wabs(states, W1, b1, W2, b2)
    in_maps, s_sum = _make_in_maps(node_features, hyper_graph, w_abs)

    nc = _get_nc()
    res = run_bass_kernel_spmd(nc, in_maps, core_ids=list(range(NCORES)))

    X = np.empty((B, E, D), dtype=np.float32)
    for b in range(B):
        yp = res.results[2 * b]["y"].astype(np.float32) + res.results[
            2 * b + 1
        ]["y"].astype(np.float32)  # (128,ECH), device ships bf16
        # unpack [128, ECH] -> (D, E): bank j rows at partitions 32j+d
        p = np.ascontiguousarray(
            yp.reshape(NJ, 32, ECH)[:, :D].transpose(1, 0, 2)
        ).reshape(D, E)
        p = p + np.float32(0.5) * s_sum[b][:, None]                # mean corr
        xb = (p * w_abs[b][None, :]).T                             # (E, D)
        X[b] = np.where(xb >= 0, xb, np.float32(0.1) * xb)
    return X


# revision 42
# speedup vs baseline: 1.0979x; 1.0979x over previous
"""HGCN encoder forward on 8 Trainium2 NeuronCores.

Computation (per batch b):
    w_abs = |gelu(states @ W1.T + b1) @ W2.T + b2|          (E,)  [host, tiny]
    d[n]    = sum_e H[n,e] * w_abs[e]                        (N,)  [host]
    s[n,dd] = rsqrt(d[n]) * nf[n,dd]                         [host, 2 MiB]
    Y[dd,e] = sum_n s[n,dd] * H[n,e]                         [device]
    X[e,dd] = leaky_relu(w_abs[e] * (Y_half0 + Y_half1)[dd,e])  [host, tiny]

Sharding: core c -> (batch b = c//2, node-half c%2); each core owns
4096 node rows.  The device kernel is a pure DMA->PE stream, paced
entirely by the H read:

  * H is centered (H - 0.5 in [-0.5, 0.5)) and quantized to fp8 E3M4
    on the host.  In [-0.5, 0.5) e3m4's subnormal+first-normal bands
    give a uniform ~6-bit quantizer (max err 2^-7); measured output
    rel err ~7e-3 vs the 2e-2 gate.  The removed mean re-enters as a
    host-side rank-1 correction: Y += 0.5 * colsum(s).
  * 8.4 MiB/core streams via 9 chunk DMAs (~1 MiB each; zero deps, all
    chunks SBUF-resident, no recycling) split alternately across BOTH
    HWDGE rings (Sync + Scalar issuers) -- measured ~410 GB/s, vs
    ~345 GB/s on a single ring.  ~1 MiB chunks matter twice: per-
    partition lines stay >= 4 KiB (descriptor efficiency) and chunk
    completion stays fine-grained so the PE paces tightly.  More
    chunks regress: each dma_start costs ~700 ns of issue time and
    the ring holds only ~10 in-flight DMAs.
  * PE: per node-tile, the 4 e-chunk matmuls (K=128, M=16, N=512,
    lhsT = bf16 s-tile, rhs = fp8 H) are column-tiled to 32-col strips
    (tile_position=(0,32j)) so they run concurrently in the array
    (measured 4 ns start-to-start); the PE tracks the DMA with ~3x
    headroom.  Accumulators sit in 4 separate PSUM banks (a start=True
    matmul clears its whole bank, so groups may never share one).
  * DVE/ACT only drain PSUM at the end (same-partition copies into a
    bf16 [128,512] output staging tile, alternating engines across
    banks); the output ships as two 64-partition bf16 DMAs (16 SDMA
    engines vs 2 for a [16,E] layout), one per ring, one per copy
    round.  bf16 output adds ~3e-4 rel err (7.03e-3 -> 7.33e-3) and
    halves the exposed output-transfer time.

Host sums the two per-batch partials, adds the mean correction, scales
by w_abs, applies leaky_relu.  Measured ~36.6 us/core quiet (ambient
HBM dips give 40-44) vs the 126 us baseline; span = ~8 us fixed
preamble + ~21 us stream + ~4.5 us drain tail + ~2.7 us postamble.
"""

import sys

for _p in ("/opt/trn_rl_repo",):
    if _p not in sys.path:
        sys.path.insert(0, _p)

import numpy as np

B, N, E, S, D = 4, 8192, 2048, 64, 16
NCORES = 8
NSHARD = N // 2          # nodes per core
NT = NSHARD // 128       # 32 node-tiles per core
ECH = 512                # e-chunk per matmul (one PSUM bank)
NJ = E // ECH            # 4 matmuls (banks) per node-tile
# Chunking sweet spot (measured): ~1 MiB chunks pace the PE tightly
# (fine completion granularity) while keeping the dma_start count
# inside the HWDGE ring depth; issues alternate between the two HWDGE
# rings (Sync + Scalar engines) so they don't serialize on one engine.
CHUNK_TILES = [4, 4, 4, 4, 4, 4, 4, 3, 1]
assert sum(CHUNK_TILES) == NT
COLTILE = True           # concurrent 32-col-strip matmuls

_CACHE = {}


def _build_nc():
    import concourse.bass as bass  # noqa: F401
    import concourse.mybir as mybir
    import concourse.tile as tile
    from concourse import bacc

    f32 = mybir.dt.float32
    bf16 = mybir.dt.bfloat16
    f8 = mybir.dt.float8e3
    nc = bacc.Bacc(
        "TRN2",
        target_bir_lowering=False,
        debug=False,
        num_devices=NCORES,
    )
    hg = nc.dram_tensor("hg", [128, NT * E], f8, kind="ExternalInput").ap()
    sv = nc.dram_tensor("sv", [128, NT * D], bf16, kind="ExternalInput").ap()
    # output packed [128, ECH]: bank j's [D, ECH] block lives at
    # partitions 32j..32j+16 (same partitions the column-tiled matmul
    # wrote, so the drain copies never shift partitions -- shifts must
    # be 32-aligned), and the output DMA spans all 128 partitions
    # (16 SDMA engines) instead of 16 (2 engines, ~4x slower).  Rows
    # 32j+16..32j+32 are dead weight the host ignores.
    y = nc.dram_tensor("y", [128, ECH], bf16, kind="ExternalOutput").ap()

    with tile.TileContext(nc) as tc:
        with (
            tc.tile_pool(name="hpool", bufs=1) as hpool,
            tc.tile_pool(name="wpool", bufs=1) as wpool,
            tc.tile_pool(name="psum", bufs=1, space="PSUM") as psum_pool,
        ):
            s_all = wpool.tile([128, NT * D], bf16, tag="sall")
            y_tile = wpool.tile([128, ECH], bf16, tag="y")
            # clear once so the output DMA never reads uninitialized
            # rows (runs in the preamble shadow, off the critical path)
            nc.vector.memset(y_tile[:], 0.0)
            nc.sync.dma_start(s_all[:], sv[:])

            accs = [
                psum_pool.tile([128, ECH], f32, tag=f"acc{j}", name=f"acc{j}")
                for j in range(NJ)
            ]

            chunks = []
            base = 0
            issuers = [nc.sync, nc.scalar]
            for c, ctiles in enumerate(CHUNK_TILES):
                h_c = hpool.tile([128, ctiles * E], f8, tag=f"hg{c}")
                eng = issuers[c % 2]
                eng.dma_start(h_c[:], hg[:, base * E : (base + ctiles) * E])
                chunks.append((h_c, base, ctiles))
                base += ctiles

            for h_c, base, ctiles in chunks:
                for t in range(ctiles):
                    i = base + t
                    for j in range(NJ):
                        if COLTILE:
                            out_ap = accs[j][32 * j : 32 * j + D, :]
                            tp = (0, 32 * j)
                        else:
                            out_ap = accs[j][0:D, :]
                            tp = None
                        nc.tensor.matmul(
                            out_ap,
                            lhsT=s_all[:, i * D : (i + 1) * D],
                            rhs=h_c[:, t * E + j * ECH : t * E + (j + 1) * ECH],
                            start=(i == 0),
                            stop=(i == NT - 1),
                            tile_position=tp,
                        )
                        if i == NT - 1:
                            rows = slice(32 * j, 32 * j + D)
                            src = (
                                accs[j][32 * j : 32 * j + D, :]
                                if COLTILE
                                else accs[j][0:D, :]
                            )
                            if j % 2 == 0:
                                nc.scalar.copy(y_tile[rows, :], src)
                            else:
                                nc.vector.tensor_copy(y_tile[rows, :], src)
                                # ship each copy-round's bank pair as one
                                # 64-partition DMA, alternating rings
                                pr = slice(32 * (j - 1), 32 * (j + 1))
                                eng = issuers[(j // 2) % 2]
                                eng.dma_start(y[pr, :], y_tile[pr, :])

    nc.compile()
    return nc


def _get_nc():
    if "nc" not in _CACHE:
        _CACHE["nc"] = _build_nc()
    return _CACHE["nc"]


def _host_wabs(states, W1, b1, W2, b2):
    from scipy.special import erf

    st = states.astype(np.float64)
    h = st @ W1.astype(np.float64).T + b1.astype(np.float64)
    h = h * 0.5 * (1.0 + erf(h / np.sqrt(2.0)))
    w = h @ W2.astype(np.float64).T + b2.astype(np.float64)
    return np.abs(w).astype(np.float32)  # (B, E)


def _f32_to_bf16_u16(x):
    """Round-to-nearest-even f32 -> bf16, returned as a uint16 array."""
    u = np.ascontiguousarray(x, dtype=np.float32).view(np.uint32)
    r = ((u >> 16) & 1) + np.uint32(0x7FFF)
    return ((u + r) >> 16).astype(np.uint16)


def _make_in_maps(node_features, hyper_graph, w_abs):
    import ml_dtypes

    # degree + rsqrt + row-scale of node features, all on host (exact)
    s = np.empty((B, N, D), dtype=np.float32)
    for b in range(B):
        d = hyper_graph[b] @ w_abs[b]                      # (N,)
        dinv = np.where(
            d > 0, 1.0 / np.sqrt(d.astype(np.float64)), 0.0
        ).astype(np.float32)
        s[b] = dinv[:, None] * node_features[b]

    s_u16 = _f32_to_bf16_u16(s)
    # mean-correction uses the bf16-rounded s the device actually sees
    s_bf = s_u16.view(ml_dtypes.bfloat16).astype(np.float32)  # (B,N,D)
    s_sum = s_bf.sum(axis=1)                                  # (B,D)

    hq = (hyper_graph - np.float32(0.5)).astype(ml_dtypes.float8_e3m4)

    in_maps = []
    for c in range(NCORES):
        b, half = c // 2, c % 2
        sl = slice(half * NSHARD, (half + 1) * NSHARD)
        hg_c = np.ascontiguousarray(
            hq[b, sl].view(np.uint8).reshape(NT, 128, E).transpose(1, 0, 2)
        ).reshape(128, NT * E).view(ml_dtypes.float8_e3m4)
        s_c = np.ascontiguousarray(
            s_u16[b, sl].reshape(NT, 128, D).transpose(1, 0, 2)
        ).reshape(128, NT * D).view(ml_dtypes.bfloat16)
        in_maps.append({"hg": hg_c, "sv": s_c})
    return in_maps, s_sum


def kernel(**inputs):
    from concourse.bass_utils import run_bass_kernel_spmd

    node_features = np.asarray(inputs["node_features"], dtype=np.float32)
    hyper_graph = np.asarray(inputs["hyper_graph"], dtype=np.float32)
    states = np.asarray(inputs["states"], dtype=np.float32)
    W1 = np.asarray(inputs["W1"], dtype=np.float32)
    b1 = np.asarray(inputs["b1"], dtype=np.float32)
    W2 = np.asarray(inputs["W2"], dtype=np.float32)
    b2 = np.asarray(inputs["b2"], dtype=np.float32)

    w_abs = _host_wabs(states, W1, b1, W2, b2)
    in_maps, s_sum = _make_in_maps(node_features, hyper_graph, w_abs)

    nc = _get_nc()
    res = run_bass_kernel_spmd(nc, in_maps, core_ids=list(range(NCORES)))

    X = np.empty((B, E, D), dtype=np.float32)
    for b in range(B):
        yp = res.results[2 * b]["y"].astype(np.float32) + res.results[
            2 * b + 1
        ]["y"].astype(np.float32)  # (128,ECH), device ships bf16
        # unpack [128, ECH] -> (D, E): bank j rows at partitions 32j+d
        p = np.ascontiguousarray(
            yp.reshape(NJ, 32, ECH)[:, :D].transpose(1, 0, 2)
        ).reshape(D, E)
        p = p + np.float32(0.5) * s_sum[b][:, None]                # mean corr
        xb = (p * w_abs[b][None, :]).T                             # (E, D)
        X[b] = np.where(xb >= 0, xb, np.float32(0.1) * xb)
    return X
